# revision 16
# baseline (speedup 1.0000x reference)
"""CrossAttn-UGCA fused kernel for Trainium2.

Data-parallel over batch: B=32 rows split across 8 NeuronCores (4 rows/core).
All weights replicated; one SPMD Bass/Tile program, per-core inputs differ.

Host-side prep (cheap numpy): transpose kv_tokens to feature-major, fold the
query-side gate term and the q-projection (tiny GEMMs) into per-row biases /
a block-diagonal per-head query operand, pack per-partition bias layouts.

Device pipeline per batch row b (all matmuls fp32r = full-rate fp32):
  feats:   absd = |kvT - q|, prod = kvT*q            (DVE, feature-major)
  gate:    h = relu(W1r.T @ [kvT; absd; prod] + gb)  (PE + ACT)
  gate2:   e = softplus(Wg2.T @ h + 2)               (PE, one-hot col b)
  chain:   g = clip(e/(e+1+1e-8), .01, .99); bias = ln g - mean + mask(-60)
  K-proj:  kT = Wk.T @ kvT + bk (feature-major)      (PE + ACT)
  scores:  s = qbd.T @ kT  (raw, pre-bias)           (PE)
  softmax: a = exp(s + bias)/sum  (no max-sub: scores bounded, mask -60)
  V-proj:  v_tok = kvT.T @ Wv + bv (token-major)     (PE + DVE)
  ctx:     c = aT.T @ v_tok, block-diag head extract (PE + ACT)
  out:     fused = LayerNorm(q + Wo.T @ c + bo)      (PE + DVE/ACT)
Outputs per core: fused [4,768], gtok [4,1024]; host concatenates.
"""

import sys

_TRN_REPO = "/opt/trn_rl_repo"
if _TRN_REPO not in sys.path:
    sys.path.insert(0, _TRN_REPO)

from contextlib import ExitStack

import numpy as np

import concourse.bass as bass
import concourse.mybir as mybir
import concourse.tile as tile
from concourse import bass_utils
from concourse.masks import make_identity
from concourse.vector_clock import ScopedClock

D, H, HD, L, B = 768, 8, 96, 1024, 32
NCORES = 8
BPC = B // NCORES          # batch rows per core
JD = D // 128              # 6 feature chunks of 128
LN_EPS = 1e-5
MASK_NEG = -60.0           # exp(-60+eps) ~ 1e-26: same as -inf after softmax

AF = mybir.ActivationFunctionType
OP = mybir.AluOpType
F32 = mybir.dt.float32
F32R = mybir.dt.float32  # plain fp32: fp32r unsupported by this toolchain
X = mybir.AxisListType.X


def _install_drain_patch():
    """This walrus build rejects instructions carrying >1 sync-wait on the
    Tile end-of-kernel drain; split the waits across extra drain instrs."""
    if getattr(tile.TileContext, "_drain_split_patched", False):
        return

    def _split_all_waits(nc):
        # Walk every block; any instruction carrying >1 sync-waits gets
        # same-engine NOPs inserted before it, each carrying one wait.
        for bb in nc.m.functions[0].blocks:
            insts = list(bb.instructions)
            out = []
            for inst in insts:
                si = getattr(inst, "sync_info", None)
                if si is not None and si.on_wait and len(si.on_wait) > 1:
                    extra, keep = list(si.on_wait[:-1]), [si.on_wait[-1]]
                    for w in extra:
                        nop = nc.engines[inst.engine].nop().ins
                        # nop() appended itself to the current bb; reclaim it
                        cur = nc.cur_bb.bb if nc.cur_bb else None
                        if cur is not None and cur.instructions and                                 cur.instructions[-1] is nop:
                            cur.instructions.pop()
                        nop.sync_info = mybir.SyncInfo(on_wait=[w], on_update=[])
                        out.append(nop)
                    si.on_wait = keep
                out.append(inst)
            bb.instructions[:] = out

    def _patched(self, tick_clock, wait_clock):
        nc = self.nc
        _split_all_waits(nc)
        drain_inst = nc.sync.drain()
        wait_clock.add_sem_waits(
            drain_inst.ins, ScopedClock({None: tick_clock.global_clock})
        )
        si = drain_inst.ins.sync_info
        if si is not None and si.on_wait and len(si.on_wait) > 1:
            waits = list(si.on_wait)
            si.on_wait = waits[:1]
            for w in waits[1:]:
                d = nc.sync.drain()
                d.ins.sync_info = mybir.SyncInfo(on_wait=[w], on_update=[])
        nc.all_engine_barrier()
        popped = nc._tile_sem_poison_stack.pop()
        assert popped is self._sem_poison
        nc.clear_and_free_semaphores(list(self.sems.allocated().values()))
        nc.all_engine_barrier()

    tile.TileContext._drain_and_barrier = _patched
    tile.TileContext._drain_split_patched = True


def _emit(nc):
    def din(name, shape):
        return nc.dram_tensor(name, shape, F32, kind="ExternalInput").ap()

    def din_r(name, shape):
        return nc.dram_tensor(name, shape, F32R, kind="ExternalInput").ap()

    def dout(name, shape):
        return nc.dram_tensor(name, shape, F32, kind="ExternalOutput").ap()

    kvt_d = din("kvt", [BPC, JD, 128, L])
    qpart_d = din("qpart", [BPC, 128, JD])
    qbd_d = din("qbd", [BPC, JD, 128, H])
    gbias_d = din("gbias", [BPC, 128, 2])
    maskb_d = din("maskb", [BPC, L])
    qres_d = din("qres", [BPC, D])
    wg1_d = din("wg1", [3 * JD, 128, 192])
    wg2oh_d = din("wg2oh", [128, 2, BPC, BPC])
    wk_d = din("wk", [JD, 128, D])
    bkp_d = din("bkp", [128, JD])
    wv_d = din("wv", [JD, 128, D])
    wo_d = din("wo", [H, HD, D])
    lng_d = din("lng", [BPC, D])
    lnb_d = din("lnb", [BPC, D])
    fused_d = dout("fused", [BPC, D])
    gtok_d = dout("gtok", [BPC, L])

    with tile.TileContext(nc) as tc, ExitStack() as ctx:
        singles = ctx.enter_context(tc.tile_pool(name="singles", bufs=1))
        kvp = ctx.enter_context(tc.tile_pool(name="kvp", bufs=2))
        featp = ctx.enter_context(tc.tile_pool(name="featp", bufs=2))
        hp = ctx.enter_context(tc.tile_pool(name="hp", bufs=1))
        ktp = ctx.enter_context(tc.tile_pool(name="ktp", bufs=2))
        vtp = ctx.enter_context(tc.tile_pool(name="vtp", bufs=2))
        sp = ctx.enter_context(tc.tile_pool(name="sp", bufs=2))
        ps = ctx.enter_context(tc.tile_pool(name="ps", bufs=4, space="PSUM"))

        _psn = [0]

        def pstile(shape):
            _psn[0] += 1
            return ps.tile(shape, F32, tag="ps", name=f"pst{_psn[0]}")

        # ---- one-time loads (weights, constants) ----
        wg1_sb = singles.tile([128, 3 * JD, 192], F32, tag="wg1")
        nc.sync.dma_start(wg1_sb, wg1_d.rearrange("c p m -> p c m"))
        wg2oh_sb = singles.tile([128, 2, BPC, BPC], F32, tag="wg2oh")
        nc.sync.dma_start(wg2oh_sb, wg2oh_d)
        wk_sb = singles.tile([128, JD, D], F32, tag="wk")
        nc.sync.dma_start(wk_sb, wk_d.rearrange("j p m -> p j m"))
        wv_sb = singles.tile([128, JD, D], F32, tag="wv")
        nc.sync.dma_start(wv_sb, wv_d.rearrange("j p m -> p j m"))
        wo_sb = singles.tile([HD, H, D], F32, tag="wo")
        nc.sync.dma_start(wo_sb, wo_d.rearrange("h p m -> p h m"))
        bkp_sb = singles.tile([128, JD], F32, tag="bkp")
        nc.sync.dma_start(bkp_sb, bkp_d)
        lng_sb = singles.tile([BPC, D], F32, tag="lng")
        nc.sync.dma_start(lng_sb, lng_d)
        lnb_sb = singles.tile([BPC, D], F32, tag="lnb")
        nc.sync.dma_start(lnb_sb, lnb_d)
        qres_sb = singles.tile([BPC, D], F32, tag="qres")
        nc.sync.dma_start(qres_sb, qres_d)
        maskb_sb = singles.tile([BPC, L], F32, tag="maskb")
        nc.sync.dma_start(maskb_sb, maskb_d)
        ident = singles.tile([8, 8], F32, tag="ident")
        make_identity(nc, ident)
        eps_sb = singles.tile([BPC, 1], F32, tag="eps")
        nc.vector.memset(eps_sb, LN_EPS)
        bg2_sb = singles.tile([BPC, 1], F32, tag="bg2")
        nc.vector.memset(bg2_sb, 2.0)
        ones8 = singles.tile([1, 8], F32, tag="ones8")
        nc.vector.memset(ones8, 1.0)

        chA = singles.tile([BPC, L], F32, tag="chA")    # e, then log g
        chB = singles.tile([BPC, L], F32, tag="chB")    # denom/recip, then g
        lsum = singles.tile([BPC, 1], F32, tag="lsum")
        nmu = singles.tile([BPC, 1], F32, tag="nmu")
        ctxcat = singles.tile([HD, H, BPC], F32, tag="ctxcat")
        x_all = singles.tile([BPC, D], F32, tag="x_all")
        fin = singles.tile([BPC, D], F32, tag="fin")
        xs = singles.tile([BPC, 1], F32, tag="xs")
        ssq = singles.tile([BPC, 1], F32, tag="ssq")
        rstd = singles.tile([BPC, 1], F32, tag="rstd")

        pse = pstile([BPC, L])

        # ---------- phase 1: gate MLP for every b ----------
        for b in range(BPC):
            kvt_b = kvp.tile([128, JD, L], F32, tag="kvt")
            nc.sync.dma_start(kvt_b, kvt_d[b].rearrange("j p f -> p j f"))
            qp_b = sp.tile([128, JD], F32, tag="qpart")
            nc.sync.dma_start(qp_b, qpart_d[b])
            gb_b = sp.tile([128, 2], F32, tag="gbias")
            nc.sync.dma_start(gb_b, gbias_d[b])

            pg = [pstile([128, L]) for _ in range(2)]
            h_b = hp.tile([128, 2, L], F32, tag="h")
            for j in range(JD):
                for nh in range(2):
                    nsl = slice(nh * 512, (nh + 1) * 512)
                    absd_jn = featp.tile([128, 512], F32, tag="absd")
                    nc.vector.tensor_scalar_sub(
                        absd_jn, kvt_b[:, j, nsl], qp_b[:, j : j + 1]
                    )
                    nc.scalar.activation(absd_jn, absd_jn, AF.Abs)
                    prod_jn = featp.tile([128, 512], F32, tag="prod")
                    nc.vector.tensor_scalar_mul(
                        prod_jn, kvt_b[:, j, nsl], qp_b[:, j : j + 1]
                    )
                    srcs = [kvt_b[:, j, nsl], absd_jn, prod_jn]
                    for m2 in range(2):
                        mw = 128 if m2 == 0 else 64
                        lsl = slice(m2 * 128, m2 * 128 + mw)
                        for si_, src in enumerate(srcs):
                            nc.tensor.matmul(
                                pg[m2][:mw, nsl],
                                wg1_sb[:, si_ * JD + j, lsl],
                                src,
                                start=(j == 0 and si_ == 0),
                                stop=(j == JD - 1 and si_ == 2),
                            )
            for m2 in range(2):
                mw = 128 if m2 == 0 else 64
                nc.scalar.activation(
                    h_b[:mw, m2, :], pg[m2][:mw, :], AF.Relu,
                    bias=gb_b[:mw, m2 : m2 + 1],
                )
            for nh in range(2):
                nsl = slice(nh * 512, (nh + 1) * 512)
                nc.tensor.matmul(
                    pse[:, nsl],
                    wg2oh_sb[:, 0, :, b],
                    h_b[:, 0, nsl],
                    start=(b == 0), stop=False,
                )
                nc.tensor.matmul(
                    pse[:, nsl],
                    wg2oh_sb[:64, 1, :, b],
                    h_b[:64, 1, nsl],
                    start=False, stop=(b == BPC - 1),
                )

        # ---------- phase 2: gating chain (all b at once) ----------
        # softplus(x) = ln(1 + exp(x)); x = pse + 2 is bounded (~2 +/- 2)
        nc.vector.tensor_scalar_add(chB, pse, 2.0)
        nc.scalar.activation(chA, chB, AF.Exp)
        nc.vector.tensor_scalar_add(chB, chA, 1.0)
        nc.scalar.activation(chA, chB, AF.Ln)
        nc.vector.tensor_scalar_add(chB, chA, 1.0 + 1e-8)
        nc.vector.reciprocal(chB, chB)
        nc.vector.tensor_tensor(chB, chA, chB, op=OP.mult)
        nc.vector.tensor_scalar_min(chB, chB, 0.99)
        nc.vector.tensor_scalar_max(chB, chB, 0.01)
        nc.sync.dma_start(gtok_d, chB)
        nc.scalar.activation(chA, chB, AF.Ln)
        nc.vector.reduce_sum(lsum, chA, axis=X)
        nc.vector.tensor_scalar_mul(nmu, lsum, -1.0 / L)
        bias_m = chB
        nc.vector.tensor_scalar_add(bias_m, chA, nmu)
        nc.vector.tensor_tensor(bias_m, bias_m, maskb_sb, op=OP.add)

        # ---------- phase 3: attention per b ----------
        for b in range(BPC):
            kvt_b = kvp.tile([128, JD, L], F32, tag="kvt")
            nc.sync.dma_start(kvt_b, kvt_d[b].rearrange("j p f -> p j f"))
            qbd_b = sp.tile([128, JD, H], F32, tag="qbd")
            nc.sync.dma_start(qbd_b, qbd_d[b].rearrange("j p h -> p j h"))

            # K-proj + raw scores, streamed per feature chunk jm
            ps_s = pstile([8, L])
            for jm in range(JD):
                pk = pstile([128, L])
                for nh in range(2):
                    nsl = slice(nh * 512, (nh + 1) * 512)
                    for jk in range(JD):
                        nc.tensor.matmul(
                            pk[:, nsl],
                            wk_sb[:, jk, jm * 128 : (jm + 1) * 128],
                            kvt_b[:, jk, nsl],
                            start=(jk == 0), stop=(jk == JD - 1),
                        )
                kt_c = ktp.tile([128, L], F32, tag="kt")
                nc.vector.tensor_scalar_add(kt_c, pk, bkp_sb[:, jm : jm + 1])
                for nh in range(2):
                    nsl = slice(nh * 512, (nh + 1) * 512)
                    nc.tensor.matmul(
                        ps_s[:, nsl],
                        qbd_b[:, jm, :],
                        kt_c[:, nsl],
                        start=(jm == 0), stop=False,
                    )
            # bias add as a K=1 ones-row matmul closing the group (broadcasts
            # the per-token bias row across the 8 head partitions). The row
            # must sit at partition 0 to match the lhsT base: DMA-stage it.
            brow = sp.tile([1, L], F32, tag="brow")
            nc.sync.dma_start(brow, bias_m[b : b + 1, :])
            for nh in range(2):
                nsl = slice(nh * 512, (nh + 1) * 512)
                nc.tensor.matmul(
                    ps_s[:, nsl],
                    ones8[0:1, :],
                    brow[0:1, nsl],
                    start=False, stop=True,
                )
            attn_sb = hp.tile([8, L], F32, tag="attn")
            asum = sp.tile([8, 1], F32, tag="asum")
            nc.scalar.activation(attn_sb, ps_s, AF.Exp)
            nc.vector.reduce_sum(asum, attn_sb, axis=X)
            arec = sp.tile([8, 1], F32, tag="arec")
            nc.vector.reciprocal(arec, asum)
            nc.vector.tensor_scalar_mul(attn_sb, attn_sb, arec)

            # transpose attention: [8, 1024] -> [128, 8(t), 8(h)] token-major
            ps_tt = pstile([128, 8, 8])
            for t in range(8):
                nc.tensor.transpose(
                    ps_tt[:, t, :], attn_sb[:, t * 128 : (t + 1) * 128], ident
                )
            attnT = sp.tile([128, 8, 8], F32, tag="attnT")
            nc.vector.tensor_copy(attnT, ps_tt)

            # V-proj (token-major) + ctx accumulate
            ps_c = pstile([8, L])
            for t in range(8):
                pv = pstile([128, L])
                tsl = slice(t * 128, (t + 1) * 128)
                for jk in range(JD):
                    nc.tensor.matmul(
                        pv[:, 0:384],
                        kvt_b[:, jk, tsl],
                        wv_sb[:, jk, 0:384],
                        start=(jk == 0), stop=(jk == JD - 1),
                    )
                    nc.tensor.matmul(
                        pv[:, 512:896],
                        kvt_b[:, jk, tsl],
                        wv_sb[:, jk, 384:768],
                        start=(jk == 0), stop=(jk == JD - 1),
                    )
                vt = vtp.tile([128, D], F32, tag="vt")
                nc.vector.tensor_copy(vt[:, 0:384], pv[:, 0:384])
                nc.vector.tensor_copy(vt[:, 384:768], pv[:, 512:896])
                nc.tensor.matmul(
                    ps_c[:, 0:384],
                    attnT[:, t, :], vt[:, 0:384],
                    start=(t == 0), stop=(t == 7),
                )
                nc.tensor.matmul(
                    ps_c[:, 512:896],
                    attnT[:, t, :], vt[:, 384:768],
                    start=(t == 0), stop=(t == 7),
                )
            # block-diagonal head extract -> ctx_row [8, 96].
            # PSUM/engine partition bases must be 32-aligned, so stage the
            # full [8, 896] to SBUF and gather per-head slices with DMAs
            # (DMA has no partition-base restriction).
            ctxf = sp.tile([8, L], F32, tag="ctxf")
            nc.scalar.copy(ctxf, ps_c)
            ctx_row = sp.tile([8, HD], F32, tag="ctx_row")
            for h in range(H):
                off = h * HD if h < 4 else 512 + (h - 4) * HD
                nc.sync.dma_start(
                    ctx_row[h : h + 1, :], ctxf[h : h + 1, off : off + HD]
                )
            ps_ct = pstile([HD, 8])
            nc.tensor.transpose(ps_ct, ctx_row, ident)
            nc.vector.tensor_copy(ctxcat[:, :, b], ps_ct)

        # ---------- phase 4: out-proj + residual + LayerNorm ----------
        ps_x = pstile([BPC, L])
        for h in range(H):
            nc.tensor.matmul(
                ps_x[:, 0:384], ctxcat[:, h, :],
                wo_sb[:, h, 0:384],
                start=(h == 0), stop=(h == H - 1),
            )
            nc.tensor.matmul(
                ps_x[:, 512:896], ctxcat[:, h, :],
                wo_sb[:, h, 384:768],
                start=(h == 0), stop=(h == H - 1),
            )
        nc.vector.tensor_tensor(
            x_all[:, 0:384], ps_x[:, 0:384], qres_sb[:, 0:384], op=OP.add
        )
        nc.vector.tensor_tensor(
            x_all[:, 384:768], ps_x[:, 512:896], qres_sb[:, 384:768], op=OP.add
        )
        nc.vector.reduce_sum(xs, x_all, axis=X)
        nc.vector.tensor_scalar_mul(nmu, xs, -1.0 / D)
        nc.vector.tensor_scalar_add(x_all, x_all, nmu)
        nc.vector.tensor_tensor(fin, x_all, x_all, op=OP.mult)
        nc.vector.reduce_sum(ssq, fin, axis=X)
        nc.vector.tensor_scalar_mul(ssq, ssq, 1.0 / D)
        nc.scalar.activation(ssq, ssq, AF.Sqrt, bias=eps_sb)
        nc.vector.reciprocal(rstd, ssq)
        nc.vector.tensor_scalar_mul(fin, x_all, rstd)
        nc.vector.tensor_tensor(fin, fin, lng_sb, op=OP.mult)
        nc.vector.tensor_tensor(fin, fin, lnb_sb, op=OP.add)
        nc.sync.dma_start(fused_d, fin)


_NC_CACHE = None


def _build():
    global _NC_CACHE
    if _NC_CACHE is None:
        _install_drain_patch()
        nc = bass.Bass("TRN2", target_bir_lowering=False, debug=False, num_devices=1)
        _emit(nc)
        _NC_CACHE = nc
    return _NC_CACHE


def _prep_inputs(q_vec, kv_tokens, kv_mask, Wg1, bg1, Wg2, bg2,
                 Wq, bq, Wk, bk, Wv, bv, Wo, bo, ln_g, ln_b):
    f32 = np.float32
    q = np.asarray(q_vec, f32)
    kv = np.asarray(kv_tokens, f32)
    mask = np.asarray(kv_mask)
    Wg1 = np.asarray(Wg1, f32)
    bg1 = np.asarray(bg1, f32)
    Wg2 = np.asarray(Wg2, f32)
    Wq = np.asarray(Wq, f32)
    bq = np.asarray(bq, f32)
    Wk_ = np.asarray(Wk, f32)
    bk_ = np.asarray(bk, f32)
    Wv_ = np.asarray(Wv, f32)
    bv_ = np.asarray(bv, f32)
    Wo_ = np.asarray(Wo, f32)
    bo_ = np.asarray(bo, f32)

    kvt = np.ascontiguousarray(kv.transpose(0, 2, 1)).reshape(B, JD, 128, L)
    qpart = np.ascontiguousarray(q.reshape(B, JD, 128).transpose(0, 2, 1))
    qp = (q @ Wq + bq) * (1.0 / np.sqrt(f32(HD)))
    qbd = np.zeros((B, JD, 128, H), f32)
    d = np.arange(D)
    qbd[:, d // 128, d % 128, d // HD] = qp
    gb_full = q @ Wg1[:D] + bg1
    gbias = np.zeros((B, 128, 2), f32)
    gbias[:, :, 0] = gb_full[:, :128]
    gbias[:, :64, 1] = gb_full[:, 128:]
    maskb = np.where(mask, f32(0.0), f32(MASK_NEG)).astype(f32)
    # ctx = sum(attn * (v_raw + bv)) = ctx_raw + bv since sum(attn) = 1,
    # so bv flows through Wo into a constant residual term.
    qres = q + bo_ + (bv_ @ Wo_)
    wg1r = np.ascontiguousarray(Wg1[D:]).reshape(3 * JD, 128, 192)
    wg2oh = np.zeros((128, 2, BPC, BPC), f32)
    wg2oh[:, 0, np.arange(BPC), np.arange(BPC)] = Wg2[:128, 0][:, None]
    wg2oh[:64, 1, np.arange(BPC), np.arange(BPC)] = Wg2[128:, 0][:, None]

    shared = dict(
        wg1=wg1r, wg2oh=wg2oh,
        wk=Wk_.reshape(JD, 128, D),
        bkp=np.ascontiguousarray(bk_.reshape(JD, 128).T),
        wv=Wv_.reshape(JD, 128, D),
        wo=Wo_.reshape(H, HD, D),
        lng=np.tile(np.asarray(ln_g, f32).reshape(1, D), (BPC, 1)),
        lnb=np.tile(np.asarray(ln_b, f32).reshape(1, D), (BPC, 1)),
    )
    in_maps = []
    for c in range(NCORES):
        s = slice(c * BPC, (c + 1) * BPC)
        m = dict(shared)
        m.update(
            kvt=np.ascontiguousarray(kvt[s]),
            qpart=np.ascontiguousarray(qpart[s]),
            qbd=np.ascontiguousarray(qbd[s]),
            gbias=np.ascontiguousarray(gbias[s]),
            maskb=np.ascontiguousarray(maskb[s]),
            qres=np.ascontiguousarray(qres[s]),
        )
        in_maps.append(m)
    return in_maps


def run(trace=False, **inputs):
    nc = _build()
    in_maps = _prep_inputs(**inputs)
    res = bass_utils.run_bass_kernel_spmd(
        nc, in_maps, core_ids=list(range(NCORES)), trace=trace
    )
    fused = np.concatenate([res.results[c]["fused"] for c in range(NCORES)], axis=0)
    gtok = np.concatenate([res.results[c]["gtok"] for c in range(NCORES)], axis=0)
    return (fused, gtok), res


def kernel(**inputs):
    outs, _ = run(trace=False, **inputs)
    return outs
